# revision 30
# baseline (speedup 1.0000x reference)
"""CenterRingFormerPlus Trainium2 Bass kernel.

Sharding: data-parallel over batch — B=8 batch elements, one per NeuronCore.
The circular rolls along the sequence are per-batch-element, hence fully
core-local (no halo exchange between cores).

Per-core layout: activations are kept feature-major [D, tokens] in SBUF so
every matmul contracts on the partition dim; the rolls become free-dim column
shifts served by an 8-column circular halo on the input. Matmuls run in
float32r (~1 cycle/row on the PE at 512-wide free dim).

Algebraic folds (host-side preprocessing, exact):
  - logits = token_proj @ centers^T = t1 @ (tc_w2 @ centers^T) + tc_b2@c^T,
    so the tc_w2 [D,DC] layer never runs on device.
  - weighted = softmax(logits) @ centers is only consumed through
    [x_ring; weighted] @ {g_w, fc_w1}; fold the bottom half of each through
    the centers: fus2 @ W = x_ring @ W_top + wfm^T @ (centers @ W_bot),
    where centers @ W_bot is a tiny [4, D] host-precomputed matrix. The
    `weighted` tensor is never materialized.
This removes 3 of the 15 D^2-per-token matmul layers.

All weights/biases are pre-packed on the HOST into DRAM blobs whose layout
matches the SBUF tiles, so every weight DMA moves contiguous 4KB
per-partition rows instead of thousands of 512B gather descriptors.

Phases per core:
  in:  DMA [128tok,1024feat] blocks, PE-transpose -> xh [128, 8, 2048+8]
  A:   h1 = gelu(ring-fusion @ fr_w1 + b1)  as 7 shifted matmul accumulations
  B:   x_ring = h1 @ fr_w2 + b2
  C:   t1 = gelu(x_ring @ tc_w1 + tb1)          (weights hoisted, all tokens)
  sm:  logits = t1 @ tcw2c + tb2c -> softmax -> wfm [4, 512] per tile
  per 512-token tile: gate = sigmoid(x_ring@gw_top + wfm^T@cg + g_b);
       fc1 = gelu(x_ring@fw1_top + wfm^T@cf + fb1); fc = fc1@fc_w2 + fb2;
       out = x_ring + gate*(fc - x_ring); PE-transpose -> token-major, DMA.

For timing, `_build_nc(reps=K)` wraps the body in a hardware For_i loop:
one NEFF execution runs the kernel K times back-to-back, amortizing the
multi-ms axon-tunnel dispatch overhead out of the per-execution
measurement (equivalent to what an NTFF device profile reports).
"""
import sys, os, time
sys.path.insert(0, '/opt/trn_rl_repo')
import numpy as np

B, N, D = 8, 2048, 1024
DC = 1024
K4 = 4
TN = 512
TT = N // TN          # 4 token tiles
HALO = 4
SHIFTS = [1, -1, 0, 2, -2, 4, -4]
P = 128
NW = N + 2 * HALO

_CACHE = {}

# experiment knobs
K_ACC_BUFS = 5
K_TP_BUFS = 2
K_W8_BUFS = 2
K_WA_BUFS = 3
K_T5_BUFS = 34


def _build_nc(reps=1):
    from concourse import bacc, mybir, tile
    F32 = mybir.dt.float32
    F32R = mybir.dt.float32r
    AF = mybir.ActivationFunctionType
    from concourse.alu_op_type import AluOpType
    AX = mybir.AxisListType

    nc = bacc.Bacc("TRN2", target_bir_lowering=False, debug=False)

    q_d = nc.dram_tensor("queries", [N, D], F32, kind="ExternalInput")
    # wA_pack[mc, h, p, j, kq, m]: fr_w1 as two 14KB-per-partition DMAs/mc
    wA_d = nc.dram_tensor("wA_pack", [8, 2, P, 7, 4, P], F32R,
                          kind="ExternalInput")
    # w8_pack[wi, g, p, mg, kc, m]: fr_w2, tc_w1, fc_w2, g_w[:D], fc_w1[:D],
    # each as two 16KB-per-partition DMAs covering 4 output chunks
    w8_d = nc.dram_tensor("w8_pack", [5, 2, P, 4, 8, P], F32R,
                          kind="ExternalInput")
    bias_d = nc.dram_tensor("bias_pack", [P, 7, 8], F32, kind="ExternalInput")
    tcw2c_d = nc.dram_tensor("tcw2c_pack", [P, 8, K4], F32R,
                             kind="ExternalInput")
    tb2c_d = nc.dram_tensor("tb2c_pack", [K4, 1], F32, kind="ExternalInput")
    cgf_d = nc.dram_tensor("cgf_pack", [2, K4, D], F32R, kind="ExternalInput")
    out_d = nc.dram_tensor("out", [N, D], F32, kind="ExternalOutput")
    ident_d = nc.inline_tensor(np.eye(P, dtype=np.float32), name="ident")

    W8_B, W8_C, W8_F, W8_G, W8_H = 0, 1, 2, 3, 4
    BI = {"b1": 0, "b2": 1, "tb1": 2, "fb1": 3, "fb2": 4, "gb": 5}

    with tile.TileContext(nc) as tc:
        with (
            tc.tile_pool(name="consts", bufs=1) as cp,
            tc.tile_pool(name="t512", bufs=K_T5_BUFS) as t5,
            tc.tile_pool(name="small", bufs=2) as smp,
            tc.tile_pool(name="wfm", bufs=4) as wfmp,
            tc.tile_pool(name="ps", bufs=1, space="PSUM") as ps,
        ):
            _eng = [0]

            def dma(dst_ap, src_ap):
                eng = nc.sync if (_eng[0] % 2 == 0) else nc.scalar
                _eng[0] += 1
                eng.dma_start(dst_ap, src_ap)

            def wload(pool, src_ap, shape, name, tag, bufs=None):
                t = pool.tile(shape, F32R, name=name, tag=tag, bufs=bufs)
                dma(t[:], src_ap)
                return t

            def body():
                # consts re-loaded per iteration (cheap) so no dependency
                # crosses the For_i back-edge semaphore reset.
                ident = cp.tile([P, P], F32, name="ident", tag="ident")
                dma(ident[:], ident_d[:, :])
                bias = cp.tile([P, 6, 8], F32, name="bias", tag="bias")
                dma(bias[:], bias_d[:, 0:6, :])

                def bslice(nm, mc):
                    return bias[:, BI[nm], mc:mc + 1]

                tcw2c = cp.tile([P, 8, K4], F32R, name="tcw2c", tag="tcw2c")
                dma(tcw2c[:], tcw2c_d[:, :, :])
                tb2c = cp.tile([K4, 1], F32, name="tb2c", tag="tb2c")
                dma(tb2c[:], tb2c_d[:, :])
                cg4 = cp.tile([K4, D], F32R, name="cg4", tag="cg4")
                dma(cg4[:], cgf_d[0])
                cf4 = cp.tile([K4, D], F32R, name="cf4", tag="cf4")
                dma(cf4[:], cgf_d[1])

                h1 = [[None] * TT for _ in range(8)]
                xring = [[None] * TT for _ in range(8)]

                # ------- macro phase 1: input + ring fusion (fr_w1) -------
                with (
                    tc.tile_pool(name="xbig", bufs=1) as xp,
                    tc.tile_pool(name="wA", bufs=K_WA_BUFS) as wap,
                ):
                    xh = xp.tile([P, 8, NW], F32R, name="xh", tag="xh")
                    # input: contiguous [128,1024] token-block DMAs, then
                    # PE-transpose; 4 feature-chunks share one [128,512] PSUM
                    # tile -> single wide DVE copy.  Last token chunk first so
                    # the left (wrap) halo completes early.
                    for i in [N // P - 1] + list(range(N // P - 1)):
                        xt = smp.tile([P, D], F32, name="xtok", tag="xtok",
                                      bufs=3)
                        dma(xt[:], q_d[i * P:(i + 1) * P, :])
                        for q in range(2):
                            pst = ps.tile([P, 4 * P], F32, name="pst",
                                          tag="tp", bufs=K_TP_BUFS)
                            for c in range(4):
                                nc.tensor.transpose(
                                    pst[:, c * P:(c + 1) * P],
                                    xt[:, (4 * q + c) * P:(4 * q + c + 1) * P],
                                    ident[:])
                            nc.vector.tensor_copy(
                                xh[:, 4 * q:4 * q + 4,
                                   HALO + i * P:HALO + (i + 1) * P],
                                pst[:].rearrange("p (c m) -> p c m", c=4))
                    nc.vector.tensor_copy(xh[:, :, 0:HALO],
                                          xh[:, :, N:N + HALO])
                    nc.vector.tensor_copy(xh[:, :, N + HALO:N + 2 * HALO],
                                          xh[:, :, HALO:2 * HALO])

                    # phase A: h1 = gelu(sum_j roll(x,s_j) @ W1_j + b1)
                    for mc in range(8):
                        accs = [ps.tile([P, TN], F32, name=f"accA{t}",
                                        tag="acc", bufs=K_ACC_BUFS)
                                for t in range(TT)]
                        wt2 = [wload(wap, wA_d[mc, h], [P, 7, 4, P],
                                     name="wA", tag="wA") for h in range(2)]
                        for k8 in range(8):
                            for j, s in enumerate(SHIFTS):
                                first = (k8 == 0 and j == 0)
                                last = (k8 == 7 and j == 6)
                                for t in range(TT):
                                    nc.tensor.matmul(
                                        accs[t][:],
                                        wt2[k8 // 4][:, j, k8 % 4, :],
                                        xh[:, k8, HALO + t * TN - s:
                                           HALO + (t + 1) * TN - s],
                                        start=first, stop=last)
                        for t in range(TT):
                            h = t5.tile([P, TN], F32R, name="h1", tag="t512")
                            nc.scalar.activation(h[:], accs[t][:], AF.Gelu,
                                                 bias=bslice("b1", mc),
                                                 scale=1.0)
                            h1[mc][t] = h

                # ---------- macro phase 2 ----------
                with (
                    tc.tile_pool(name="xr", bufs=32) as xrp,
                    tc.tile_pool(name="w8", bufs=K_W8_BUFS) as w8p,
                    tc.tile_pool(name="ot", bufs=2) as otp,
                ):
                    # phase B: x_ring = h1 @ fr_w2 + b2
                    for g in range(2):
                        wg = wload(w8p, w8_d[W8_B, g], [P, 4, 8, P],
                                   name="wB", tag="w8")
                        for mg in range(4):
                            mc = g * 4 + mg
                            accs = [ps.tile([P, TN], F32, name=f"accB{t}",
                                            tag="acc", bufs=K_ACC_BUFS)
                                    for t in range(TT)]
                            for kc in range(8):
                                for t in range(TT):
                                    nc.tensor.matmul(accs[t][:],
                                                     wg[:, mg, kc, :],
                                                     h1[kc][t][:],
                                                     start=(kc == 0),
                                                     stop=(kc == 7))
                            for t in range(TT):
                                xr = xrp.tile([P, TN], F32R, name="xring",
                                              tag="xr")
                                nc.scalar.activation(xr[:], accs[t][:],
                                                     AF.Identity,
                                                     bias=bslice("b2", mc),
                                                     scale=1.0)
                                xring[mc][t] = xr

                    # phase C: t1 = gelu(x_ring @ tc_w1 + tb1), hoisted
                    t1 = [[None] * TT for _ in range(8)]
                    for g in range(2):
                        wg = wload(w8p, w8_d[W8_C, g], [P, 4, 8, P],
                                   name="wC", tag="w8")
                        for mg in range(4):
                            mc = g * 4 + mg
                            accs = [ps.tile([P, TN], F32, name=f"accC{t}",
                                            tag="acc", bufs=K_ACC_BUFS)
                                    for t in range(TT)]
                            for kc in range(8):
                                for t in range(TT):
                                    nc.tensor.matmul(accs[t][:],
                                                     wg[:, mg, kc, :],
                                                     xring[kc][t][:],
                                                     start=(kc == 0),
                                                     stop=(kc == 7))
                            for t in range(TT):
                                h = t5.tile([P, TN], F32R, name="t1",
                                            tag="t512")
                                nc.scalar.activation(h[:], accs[t][:],
                                                     AF.Gelu,
                                                     bias=bslice("tb1", mc),
                                                     scale=1.0)
                                t1[mc][t] = h

                    # softmax: logits[k, tok] = t1 @ tcw2c + tb2c (k-major,
                    # so the 8 accumulating matmuls stream 512-wide and the
                    # +tb2c lands as a per-partition Exp bias).  No max
                    # subtraction: |logit| ~ N(0,5), f32 exp is safe.
                    wfm = []
                    for t in range(TT):
                        psl2 = ps.tile([K4, TN], F32, name="psl2", tag="tp",
                                       bufs=K_TP_BUFS)
                        for kc in range(8):
                            nc.tensor.matmul(psl2[:], tcw2c[:, kc, :],
                                             t1[kc][t][:],
                                             start=(kc == 0), stop=(kc == 7))
                        e4 = smp.tile([K4, TN], F32, name="e4", tag="e4")
                        nc.scalar.activation(e4[:], psl2[:], AF.Exp,
                                             bias=tb2c[0:K4, 0:1], scale=1.0)
                        wf = wfmp.tile([K4, TN], F32R, name="wfm", tag="wfm")
                        for i4 in range(TN // P):
                            pse = ps.tile([P, K4], F32, name="pse",
                                          tag="tps", bufs=1)
                            nc.tensor.transpose(
                                pse[:], e4[0:K4, i4 * P:(i4 + 1) * P],
                                ident[0:K4, 0:K4])
                            z = smp.tile([P, 1], F32, name="zsm", tag="zsm")
                            nc.vector.reduce_sum(z[:], pse[:], AX.X)
                            rz = smp.tile([P, 1], F32, name="rz", tag="rz")
                            nc.vector.reciprocal(rz[:], z[:])
                            wtok = smp.tile([P, K4], F32, name="wtok",
                                            tag="wtok")
                            nc.vector.tensor_scalar_mul(wtok[:], pse[:],
                                                        rz[:])
                            pstw = ps.tile([K4, P], F32, name="pstw",
                                           tag="tps", bufs=1)
                            nc.tensor.transpose(pstw[:], wtok[:], ident[:])
                            nc.vector.tensor_copy(
                                wf[0:K4, i4 * P:(i4 + 1) * P], pstw[:])
                        wfm.append(wf)

                    # tail per 512-token tile: gate, fc1 (rank-4 fold), fc,
                    # gating, transpose-out
                    for t in range(TT):
                        gate, fc1 = [], []
                        for wi, c4t, bs, fn, odt, nm in (
                            (W8_G, cg4, "gb", AF.Sigmoid, F32, "gate"),
                            (W8_H, cf4, "fb1", AF.Gelu, F32R, "fc1"),
                        ):
                            dst = gate if nm == "gate" else fc1
                            for g in range(2):
                                wg = wload(w8p, w8_d[wi, g], [P, 4, 8, P],
                                           name=f"w_{nm}", tag="w8")
                                for mg in range(4):
                                    mc = g * 4 + mg
                                    acc = ps.tile([P, TN], F32, name="accG",
                                                  tag="acc", bufs=K_ACC_BUFS)
                                    for kc in range(8):
                                        nc.tensor.matmul(acc[:],
                                                         wg[:, mg, kc, :],
                                                         xring[kc][t][:],
                                                         start=(kc == 0),
                                                         stop=False)
                                    nc.tensor.matmul(
                                        acc[:],
                                        c4t[0:K4, mc * P:(mc + 1) * P],
                                        wfm[t][0:K4, :], start=False,
                                        stop=True)
                                    o = t5.tile([P, TN], odt, name=nm,
                                                tag="t512")
                                    nc.scalar.activation(o[:], acc[:], fn,
                                                         bias=bslice(bs, mc),
                                                         scale=1.0)
                                    dst.append(o)
                        # fc = fc1 @ fc_w2 + fb2
                        fc = []
                        for g in range(2):
                            wg = wload(w8p, w8_d[W8_F, g], [P, 4, 8, P],
                                       name="wF", tag="w8")
                            for mg in range(4):
                                mc = g * 4 + mg
                                acc = ps.tile([P, TN], F32, name="accF",
                                              tag="acc", bufs=K_ACC_BUFS)
                                for kc in range(8):
                                    nc.tensor.matmul(acc[:],
                                                     wg[:, mg, kc, :],
                                                     fc1[kc][:],
                                                     start=(kc == 0),
                                                     stop=(kc == 7))
                                o = t5.tile([P, TN], F32, name="fc",
                                            tag="t512")
                                nc.scalar.activation(o[:], acc[:],
                                                     AF.Identity,
                                                     bias=bslice("fb2", mc),
                                                     scale=1.0)
                                fc.append(o)
                        # gating in place: fc = x_ring + gate*(fc - x_ring)
                        for mc in range(8):
                            nc.vector.tensor_sub(fc[mc][:], fc[mc][:],
                                                 xring[mc][t][:])
                            nc.vector.tensor_mul(fc[mc][:], fc[mc][:],
                                                 gate[mc][:])
                            nc.vector.tensor_add(fc[mc][:], fc[mc][:],
                                                 xring[mc][t][:])
                        # transpose to token-major and store; 4 feature
                        # chunks share one [128,512] PSUM tile -> 2 copies
                        for i4 in range(TN // P):
                            ot = otp.tile([P, D], F32, name="ot", tag="ot")
                            for h in range(2):
                                pst = ps.tile([P, 4 * P], F32, name="psto",
                                              tag="tp", bufs=K_TP_BUFS)
                                for c in range(4):
                                    nc.tensor.transpose(
                                        pst[:, c * P:(c + 1) * P],
                                        fc[4 * h + c][:,
                                                      i4 * P:(i4 + 1) * P],
                                        ident[:])
                                nc.vector.tensor_copy(
                                    ot[:, h * 4 * P:(h + 1) * 4 * P], pst[:])
                            r0 = t * TN + i4 * P
                            # stores go via the idle GPSIMD SWDGE queue to
                            # keep the SP/Act HWDGE queues free for weights
                            nc.gpsimd.dma_start(out_d[r0:r0 + P, :], ot[:])

            if reps == 1:
                body()
            else:
                # hint_engines: body is ~3600 PE instructions (many IRAM
                # blocks), so the back-edge branch would I$-miss (~4us)
                # every iteration without the prefetch hint.
                ET = mybir.EngineType
                with tc.For_i(0, reps, 1,
                              hint_engines=(ET.PE, ET.Activation, ET.DVE,
                                            ET.SP)):
                    body()

    nc.compile()
    return nc


def _get_nc(reps=1):
    key = f"nc{reps}"
    if key not in _CACHE:
        _CACHE[key] = _build_nc(reps=reps)
    return _CACHE[key]


def _pack_inputs(inputs):
    """Host-side packing of weights into SBUF-tile-ordered DRAM blobs."""
    f = lambda n: np.asarray(inputs[n], dtype=np.float32)
    cen = f("centers")
    g_w, fc_w1 = f("g_w"), f("fc_w1")
    # wA_pack[mc, h, p, j, kq, m] = fr_w1[j*1024 + (4h+kq)*128 + p, mc*128+m]
    wA = np.ascontiguousarray(
        f("fr_w1").reshape(7, 2, 4, P, 8, P).transpose(4, 1, 3, 0, 2, 5))
    # w8_pack[wi, g, p, mg, kc, m] = W[kc*128 + p, (4g+mg)*128 + m]
    w8 = np.ascontiguousarray(np.stack(
        [w.reshape(8, P, 2, 4, P).transpose(2, 1, 3, 0, 4)
         for w in (f("fr_w2"), f("tc_w1"), f("fc_w2"),
                   g_w[:D], fc_w1[:D])]))
    bias = np.zeros((P, 7, 8), np.float32)
    for i, n in enumerate(("fr_b1", "fr_b2", "tc_b1", "fc_b1", "fc_b2",
                           "g_b")):
        bias[:, i, :] = f(n).reshape(8, P).T
    # tcw2c[p, kc, k] = (tc_w2 @ centers^T)[kc*128 + p, k]
    tcw2c = np.ascontiguousarray(
        (f("tc_w2") @ cen.T).reshape(8, P, K4).transpose(1, 0, 2))
    tb2c = np.ascontiguousarray(
        (f("tc_b2") @ cen.T).reshape(K4, 1))
    cgf = np.ascontiguousarray(
        np.stack([cen @ g_w[D:], cen @ fc_w1[D:]]))
    return {
        "wA_pack": wA, "w8_pack": w8, "bias_pack": bias,
        "tcw2c_pack": tcw2c, "tb2c_pack": tb2c, "cgf_pack": cgf,
    }


def _in_maps(inputs):
    shared = _pack_inputs(inputs)
    q = np.asarray(inputs["queries"], dtype=np.float32)
    return [dict(shared, queries=np.ascontiguousarray(q[c]))
            for c in range(B)]


def kernel(**inputs) -> np.ndarray:
    from concourse import bass_utils
    nc = _get_nc(reps=1)
    res = bass_utils.run_bass_kernel_spmd(nc, _in_maps(inputs),
                                          core_ids=list(range(B)))
    return np.stack([res.results[c]["out"] for c in range(B)], axis=0)


REPS = 250
NPIPE = 6


def kernel_timed(inputs, iters=3):
    """Returns (output [B,N,D], best_seconds_per_execution).

    Times a NEFF whose body is the full kernel repeated REPS times in a
    hardware loop; per-execution time = burst_wall / (NPIPE * REPS). This
    amortizes the axon-tunnel dispatch overhead (~1-80 ms per dispatch),
    which would otherwise dominate the measurement.
    """
    import jax
    from jax.sharding import Mesh, PartitionSpec, NamedSharding
    from jax.experimental.shard_map import shard_map
    from concourse import mybir
    from concourse.bass2jax import (_bass_exec_p, install_neuronx_cc_hook,
                                    partition_id_tensor)
    nc = _get_nc(reps=REPS)
    install_neuronx_cc_hook()
    partition_name = (nc.partition_id_tensor.name
                      if nc.partition_id_tensor else None)
    in_names, out_names, out_avals = [], [], []
    for alloc in nc.m.functions[0].allocations:
        if not isinstance(alloc, mybir.MemoryLocationSet):
            continue
        name = alloc.memorylocations[0].name
        if alloc.kind == "ExternalInput":
            if name != partition_name:
                in_names.append(name)
        elif alloc.kind == "ExternalOutput":
            out_names.append(name)
            out_avals.append(jax.core.ShapedArray(
                tuple(alloc.tensor_shape), mybir.dt.np(alloc.dtype)))

    all_in = list(in_names) + list(out_names)
    if partition_name is not None:
        all_in.append(partition_name)

    def _body(*args):
        operands = list(args)
        if partition_name is not None:
            operands.append(partition_id_tensor())
        return tuple(_bass_exec_p.bind(
            *operands, out_avals=tuple(out_avals), in_names=tuple(all_in),
            out_names=tuple(out_names), lowering_input_output_aliases=(),
            sim_require_finite=True, sim_require_nnan=True, nc=nc))

    devices = jax.devices()[:B]
    mesh = Mesh(np.asarray(devices), ("core",))
    n_par, n_out = len(in_names), len(out_names)
    fn = jax.jit(shard_map(_body, mesh=mesh,
                           in_specs=(PartitionSpec("core"),) * (n_par + n_out),
                           out_specs=(PartitionSpec("core"),) * n_out,
                           check_rep=False), keep_unused=True)
    sh = NamedSharding(mesh, PartitionSpec("core"))
    im = _in_maps(inputs)
    dev_args = [jax.device_put(
        np.concatenate([np.asarray(im[c][n]) for c in range(B)], axis=0), sh)
        for n in in_names]
    dev_zero = [jax.device_put(
        np.zeros((B * a.shape[0], *a.shape[1:]), a.dtype), sh)
        for a in out_avals]
    jax.block_until_ready(dev_args + dev_zero)
    outs = fn(*dev_args, *dev_zero)
    jax.block_until_ready(outs)
    best = None
    for _ in range(3):
        t0 = time.perf_counter()
        last = None
        for _ in range(NPIPE):
            last = fn(*dev_args, *dev_zero)
        jax.block_until_ready(last)
        wall = time.perf_counter() - t0
        per = wall / (NPIPE * REPS)
        print(f"burst: {wall*1e3:.1f} ms / {NPIPE}x{REPS} execs "
              f"= {per*1e6:.1f} us/exec", flush=True)
        best = per if best is None else min(best, per)
    oi = out_names.index("out")
    full = np.asarray(outs[oi]).reshape(B, N, D)
    return full, best


# revision 38
# speedup vs baseline: 1.4188x; 1.4188x over previous
"""CenterRingFormerPlus Trainium2 Bass kernel.

Sharding: data-parallel over batch — B=8 batch elements, one per NeuronCore.
The circular rolls along the sequence are per-batch-element, hence fully
core-local (no halo exchange between cores).

Per-core layout: activations are kept feature-major [D, tokens] in SBUF so
every matmul contracts on the partition dim; the rolls become free-dim column
shifts served by an 8-column circular halo on the input. Matmuls run in
float32r (~1 cycle/row on the PE at 512-wide free dim).

Algebraic folds (host-side preprocessing, exact):
  - logits = token_proj @ centers^T = t1 @ (tc_w2 @ centers^T) + tc_b2@c^T,
    so the tc_w2 [D,DC] layer never runs on device.
  - weighted = softmax(logits) @ centers is only consumed through
    [x_ring; weighted] @ {g_w, fc_w1}; fold the bottom half of each through
    the centers: fus2 @ W = x_ring @ W_top + wfm^T @ (centers @ W_bot),
    where centers @ W_bot is a tiny [4, D] host-precomputed matrix. The
    `weighted` tensor is never materialized.
This removes 3 of the 15 D^2-per-token matmul layers.

All weights/biases are pre-packed on the HOST into DRAM blobs whose layout
matches the SBUF tiles, so every weight DMA moves contiguous 4KB
per-partition rows instead of thousands of 512B gather descriptors.

Phases per core:
  in:  DMA [128tok,1024feat] blocks, PE-transpose -> xh [128, 8, 2048+8]
  A:   h1 = gelu(ring-fusion @ fr_w1 + b1)  as 7 shifted matmul accumulations
  B:   x_ring = h1 @ fr_w2 + b2
  C:   t1 = gelu(x_ring @ tc_w1 + tb1)          (weights hoisted, all tokens)
  sm:  logits = t1 @ tcw2c + tb2c -> softmax -> wfm [4, 512] per tile
  per 512-token tile: gate = sigmoid(x_ring@gw_top + wfm^T@cg + g_b);
       fc1 = gelu(x_ring@fw1_top + wfm^T@cf + fb1); fc = fc1@fc_w2 + fb2;
       out = x_ring + gate*(fc - x_ring); PE-transpose -> token-major, DMA.

For timing, `_build_nc(reps=K)` wraps the body in a hardware For_i loop:
one NEFF execution runs the kernel K times back-to-back, amortizing the
multi-ms axon-tunnel dispatch overhead out of the per-execution
measurement (equivalent to what an NTFF device profile reports).
"""
import sys, os, time
sys.path.insert(0, '/opt/trn_rl_repo')
import numpy as np

B, N, D = 8, 2048, 1024
DC = 1024
K4 = 4
TN = 512
TT = N // TN          # 4 token tiles
HALO = 4
SHIFTS = [1, -1, 0, 2, -2, 4, -4]
P = 128
NW = N + 2 * HALO

_CACHE = {}

# experiment knobs
K_ACC_BUFS = 5
K_TP_BUFS = 2
K_W8_BUFS = 6
K_WA_BUFS = 2
K_T5_BUFS = 36


def _build_nc(reps=1):
    from concourse import bacc, mybir, tile
    F32 = mybir.dt.float32
    F32R = mybir.dt.float32r
    AF = mybir.ActivationFunctionType
    from concourse.alu_op_type import AluOpType
    AX = mybir.AxisListType

    nc = bacc.Bacc("TRN2", target_bir_lowering=False, debug=False)

    q_d = nc.dram_tensor("queries", [N, D], F32, kind="ExternalInput")
    # wA_pack[mc, j, p, kc, m]: one contiguous [128, 4KB] DMA per (mc, j)
    wA_d = nc.dram_tensor("wA_pack", [8, 7, P, 8, P], F32R,
                          kind="ExternalInput")
    # w8_pack[wi, mc, p, kc, m]: fr_w2, tc_w1, fc_w2, g_w[:D], fc_w1[:D]
    w8_d = nc.dram_tensor("w8_pack", [5, 8, P, 8, P], F32R,
                          kind="ExternalInput")
    bias_d = nc.dram_tensor("bias_pack", [P, 7, 8], F32, kind="ExternalInput")
    tcw2c_d = nc.dram_tensor("tcw2c_pack", [P, 8, K4], F32R,
                             kind="ExternalInput")
    tb2c_d = nc.dram_tensor("tb2c_pack", [K4, 1], F32, kind="ExternalInput")
    cgf_d = nc.dram_tensor("cgf_pack", [2, K4, D], F32R, kind="ExternalInput")
    out_d = nc.dram_tensor("out", [N, D], F32, kind="ExternalOutput")
    ident_d = nc.inline_tensor(np.eye(P, dtype=np.float32), name="ident")

    W8_B, W8_C, W8_F, W8_G, W8_H = 0, 1, 2, 3, 4
    BI = {"b1": 0, "b2": 1, "tb1": 2, "fb1": 3, "fb2": 4, "gb": 5}

    with tile.TileContext(nc) as tc:
        with (
            tc.tile_pool(name="consts", bufs=1) as cp,
            tc.tile_pool(name="t512", bufs=K_T5_BUFS) as t5,
            tc.tile_pool(name="small", bufs=2) as smp,
            tc.tile_pool(name="wfm", bufs=4) as wfmp,
            tc.tile_pool(name="ps", bufs=1, space="PSUM") as ps,
        ):
            _eng = [0]

            def dma(dst_ap, src_ap):
                eng = nc.sync if (_eng[0] % 2 == 0) else nc.scalar
                _eng[0] += 1
                eng.dma_start(dst_ap, src_ap)

            def wload(pool, src_ap, shape, name, tag, bufs=None):
                t = pool.tile(shape, F32R, name=name, tag=tag, bufs=bufs)
                dma(t[:], src_ap)
                return t

            def body():
                # consts re-loaded per iteration (cheap) so no dependency
                # crosses the For_i back-edge semaphore reset.
                ident = cp.tile([P, P], F32, name="ident", tag="ident")
                dma(ident[:], ident_d[:, :])
                bias = cp.tile([P, 6, 8], F32, name="bias", tag="bias")
                dma(bias[:], bias_d[:, 0:6, :])

                def bslice(nm, mc):
                    return bias[:, BI[nm], mc:mc + 1]

                tcw2c = cp.tile([P, 8, K4], F32R, name="tcw2c", tag="tcw2c")
                dma(tcw2c[:], tcw2c_d[:, :, :])
                tb2c = cp.tile([K4, 1], F32, name="tb2c", tag="tb2c")
                dma(tb2c[:], tb2c_d[:, :])
                cg4 = cp.tile([K4, D], F32R, name="cg4", tag="cg4")
                dma(cg4[:], cgf_d[0])
                cf4 = cp.tile([K4, D], F32R, name="cf4", tag="cf4")
                dma(cf4[:], cgf_d[1])

                h1 = [[None] * TT for _ in range(8)]
                xring = [[None] * TT for _ in range(8)]

                # ------- macro phase 1: input + ring fusion (fr_w1) -------
                with (
                    tc.tile_pool(name="xbig", bufs=1) as xp,
                    tc.tile_pool(name="wA", bufs=K_WA_BUFS) as wap,
                ):
                    xh = xp.tile([P, 8, NW], F32R, name="xh", tag="xh")
                    # input: contiguous [128,1024] token-block DMAs, then
                    # PE-transpose; 4 feature-chunks share one [128,512] PSUM
                    # tile -> single wide DVE copy.  Last token chunk first so
                    # the left (wrap) halo completes early.
                    for i in [N // P - 1] + list(range(N // P - 1)):
                        xt = smp.tile([P, D], F32, name="xtok", tag="xtok",
                                      bufs=3)
                        dma(xt[:], q_d[i * P:(i + 1) * P, :])
                        for q in range(2):
                            pst = ps.tile([P, 4 * P], F32, name="pst",
                                          tag="tp", bufs=K_TP_BUFS)
                            for c in range(4):
                                nc.tensor.transpose(
                                    pst[:, c * P:(c + 1) * P],
                                    xt[:, (4 * q + c) * P:(4 * q + c + 1) * P],
                                    ident[:])
                            nc.vector.tensor_copy(
                                xh[:, 4 * q:4 * q + 4,
                                   HALO + i * P:HALO + (i + 1) * P],
                                pst[:].rearrange("p (c m) -> p c m", c=4))
                    nc.vector.tensor_copy(xh[:, :, 0:HALO],
                                          xh[:, :, N:N + HALO])
                    nc.vector.tensor_copy(xh[:, :, N + HALO:N + 2 * HALO],
                                          xh[:, :, HALO:2 * HALO])

                    # phase A: h1 = gelu(sum_j roll(x,s_j) @ W1_j + b1)
                    for mc in range(8):
                        accs = [ps.tile([P, TN], F32, name=f"accA{t}",
                                        tag="acc", bufs=K_ACC_BUFS)
                                for t in range(TT)]
                        for j, s in enumerate(SHIFTS):
                            wj = wload(wap, wA_d[mc, j], [P, 8, P],
                                       name="wA", tag="wA")
                            for k8 in range(8):
                                first = (j == 0 and k8 == 0)
                                last = (j == 6 and k8 == 7)
                                for t in range(TT):
                                    nc.tensor.matmul(
                                        accs[t][:], wj[:, k8, :],
                                        xh[:, k8, HALO + t * TN - s:
                                           HALO + (t + 1) * TN - s],
                                        start=first, stop=last)
                        for t in range(TT):
                            h = t5.tile([P, TN], F32R, name="h1", tag="t512")
                            nc.scalar.activation(h[:], accs[t][:], AF.Gelu,
                                                 bias=bslice("b1", mc),
                                                 scale=1.0)
                            h1[mc][t] = h

                # ---------- macro phase 2 ----------
                with (
                    tc.tile_pool(name="xr", bufs=32) as xrp,
                    tc.tile_pool(name="w8", bufs=K_W8_BUFS) as w8p,
                    tc.tile_pool(name="ot", bufs=2) as otp,
                ):
                    # phase B: x_ring = h1 @ fr_w2 + b2
                    for mc in range(8):
                        wcol = wload(w8p, w8_d[W8_B, mc], [P, 8, P],
                                     name="wB", tag="w8")
                        accs = [ps.tile([P, TN], F32, name=f"accB{t}",
                                        tag="acc", bufs=K_ACC_BUFS)
                                for t in range(TT)]
                        for kc in range(8):
                            for t in range(TT):
                                nc.tensor.matmul(accs[t][:], wcol[:, kc, :],
                                                 h1[kc][t][:],
                                                 start=(kc == 0),
                                                 stop=(kc == 7))
                        for t in range(TT):
                            xr = xrp.tile([P, TN], F32R, name="xring",
                                          tag="xr")
                            nc.scalar.activation(xr[:], accs[t][:],
                                                 AF.Identity,
                                                 bias=bslice("b2", mc),
                                                 scale=1.0)
                            xring[mc][t] = xr

                    # phase C: t1 = gelu(x_ring @ tc_w1 + tb1), hoisted
                    t1 = [[None] * TT for _ in range(8)]
                    for mc in range(8):
                        wcol = wload(w8p, w8_d[W8_C, mc], [P, 8, P],
                                     name="wC", tag="w8")
                        accs = [ps.tile([P, TN], F32, name=f"accC{t}",
                                        tag="acc", bufs=K_ACC_BUFS)
                                for t in range(TT)]
                        for kc in range(8):
                            for t in range(TT):
                                nc.tensor.matmul(accs[t][:], wcol[:, kc, :],
                                                 xring[kc][t][:],
                                                 start=(kc == 0),
                                                 stop=(kc == 7))
                        for t in range(TT):
                            h = t5.tile([P, TN], F32R, name="t1",
                                        tag="t512")
                            nc.scalar.activation(h[:], accs[t][:], AF.Gelu,
                                                 bias=bslice("tb1", mc),
                                                 scale=1.0)
                            t1[mc][t] = h

                    # softmax: logits[k, tok] = t1 @ tcw2c + tb2c (k-major,
                    # so the 8 accumulating matmuls stream 512-wide and the
                    # +tb2c lands as a per-partition Exp bias).  No max
                    # subtraction: |logit| ~ N(0,5), f32 exp is safe.
                    wfm = []
                    for t in range(TT):
                        psl2 = ps.tile([K4, TN], F32, name="psl2", tag="tp",
                                       bufs=K_TP_BUFS)
                        for kc in range(8):
                            nc.tensor.matmul(psl2[:], tcw2c[:, kc, :],
                                             t1[kc][t][:],
                                             start=(kc == 0), stop=(kc == 7))
                        e4 = smp.tile([K4, TN], F32, name="e4", tag="e4")
                        nc.scalar.activation(e4[:], psl2[:], AF.Exp,
                                             bias=tb2c[0:K4, 0:1], scale=1.0)
                        wf = wfmp.tile([K4, TN], F32R, name="wfm", tag="wfm")
                        for i4 in range(TN // P):
                            pse = ps.tile([P, K4], F32, name="pse",
                                          tag="tps", bufs=1)
                            nc.tensor.transpose(
                                pse[:], e4[0:K4, i4 * P:(i4 + 1) * P],
                                ident[0:K4, 0:K4])
                            z = smp.tile([P, 1], F32, name="zsm", tag="zsm")
                            nc.vector.reduce_sum(z[:], pse[:], AX.X)
                            rz = smp.tile([P, 1], F32, name="rz", tag="rz")
                            nc.vector.reciprocal(rz[:], z[:])
                            wtok = smp.tile([P, K4], F32, name="wtok",
                                            tag="wtok")
                            nc.vector.tensor_scalar_mul(wtok[:], pse[:],
                                                        rz[:])
                            pstw = ps.tile([K4, P], F32, name="pstw",
                                           tag="tps", bufs=1)
                            nc.tensor.transpose(pstw[:], wtok[:], ident[:])
                            nc.vector.tensor_copy(
                                wf[0:K4, i4 * P:(i4 + 1) * P], pstw[:])
                        wfm.append(wf)

                    # tail per 512-token tile: gate, fc1 (rank-4 fold), fc,
                    # gating, transpose-out
                    for t in range(TT):
                        gate, fc1 = [], []
                        for wi, c4t, bs, fn, odt, nm in (
                            (W8_G, cg4, "gb", AF.Sigmoid, F32, "gate"),
                            (W8_H, cf4, "fb1", AF.Gelu, F32R, "fc1"),
                        ):
                            dst = gate if nm == "gate" else fc1
                            for mc in range(8):
                                wcol = wload(w8p, w8_d[wi, mc], [P, 8, P],
                                             name=f"w_{nm}", tag="w8")
                                acc = ps.tile([P, TN], F32, name="accG",
                                              tag="acc", bufs=K_ACC_BUFS)
                                for kc in range(8):
                                    nc.tensor.matmul(acc[:], wcol[:, kc, :],
                                                     xring[kc][t][:],
                                                     start=(kc == 0),
                                                     stop=False)
                                nc.tensor.matmul(
                                    acc[:], c4t[0:K4, mc * P:(mc + 1) * P],
                                    wfm[t][0:K4, :], start=False, stop=True)
                                o = t5.tile([P, TN], odt, name=nm,
                                            tag="t512")
                                nc.scalar.activation(o[:], acc[:], fn,
                                                     bias=bslice(bs, mc),
                                                     scale=1.0)
                                dst.append(o)
                        # fc = fc1 @ fc_w2 + fb2
                        fc = []
                        for mc in range(8):
                            wcol = wload(w8p, w8_d[W8_F, mc], [P, 8, P],
                                         name="wF", tag="w8")
                            acc = ps.tile([P, TN], F32, name="accF",
                                          tag="acc", bufs=K_ACC_BUFS)
                            for kc in range(8):
                                nc.tensor.matmul(acc[:], wcol[:, kc, :],
                                                 fc1[kc][:],
                                                 start=(kc == 0),
                                                 stop=(kc == 7))
                            o = t5.tile([P, TN], F32, name="fc", tag="t512")
                            nc.scalar.activation(o[:], acc[:], AF.Identity,
                                                 bias=bslice("fb2", mc),
                                                 scale=1.0)
                            fc.append(o)
                        # gating in place: fc = x_ring + gate*(fc - x_ring)
                        for mc in range(8):
                            nc.vector.tensor_sub(fc[mc][:], fc[mc][:],
                                                 xring[mc][t][:])
                            nc.vector.tensor_mul(fc[mc][:], fc[mc][:],
                                                 gate[mc][:])
                            nc.vector.tensor_add(fc[mc][:], fc[mc][:],
                                                 xring[mc][t][:])
                        # transpose to token-major and store; 4 feature
                        # chunks share one [128,512] PSUM tile -> 2 copies
                        for i4 in range(TN // P):
                            ot = otp.tile([P, D], F32, name="ot", tag="ot")
                            for h in range(2):
                                pst = ps.tile([P, 4 * P], F32, name="psto",
                                              tag="tp", bufs=K_TP_BUFS)
                                for c in range(4):
                                    nc.tensor.transpose(
                                        pst[:, c * P:(c + 1) * P],
                                        fc[4 * h + c][:,
                                                      i4 * P:(i4 + 1) * P],
                                        ident[:])
                                nc.vector.tensor_copy(
                                    ot[:, h * 4 * P:(h + 1) * 4 * P], pst[:])
                            r0 = t * TN + i4 * P
                            # stores go via the idle GPSIMD SWDGE queue to
                            # keep the SP/Act HWDGE queues free for weights
                            nc.gpsimd.dma_start(out_d[r0:r0 + P, :], ot[:])

            if reps == 1:
                body()
            else:
                # hint_engines: body is ~3600 PE instructions (many IRAM
                # blocks), so the back-edge branch would I$-miss (~4us)
                # every iteration without the prefetch hint.
                ET = mybir.EngineType
                with tc.For_i(0, reps, 1,
                              hint_engines=(ET.PE, ET.Activation, ET.DVE,
                                            ET.SP)):
                    body()

    nc.compile()
    return nc


def _get_nc(reps=1):
    key = f"nc{reps}"
    if key not in _CACHE:
        _CACHE[key] = _build_nc(reps=reps)
    return _CACHE[key]


def _pack_inputs(inputs):
    """Host-side packing of weights into SBUF-tile-ordered DRAM blobs."""
    f = lambda n: np.asarray(inputs[n], dtype=np.float32)
    cen = f("centers")
    g_w, fc_w1 = f("g_w"), f("fc_w1")
    # wA_pack[mc, j, p, kc, m] = fr_w1[j*1024 + kc*128 + p, mc*128 + m]
    wA = np.ascontiguousarray(
        f("fr_w1").reshape(7, 8, P, 8, P).transpose(3, 0, 2, 1, 4))
    # w8_pack[wi, mc, p, kc, m] = W[kc*128 + p, mc*128 + m]
    w8 = np.ascontiguousarray(np.stack(
        [w.reshape(8, P, 8, P).transpose(2, 1, 0, 3)
         for w in (f("fr_w2"), f("tc_w1"), f("fc_w2"),
                   g_w[:D], fc_w1[:D])]))
    bias = np.zeros((P, 7, 8), np.float32)
    for i, n in enumerate(("fr_b1", "fr_b2", "tc_b1", "fc_b1", "fc_b2",
                           "g_b")):
        bias[:, i, :] = f(n).reshape(8, P).T
    # tcw2c[p, kc, k] = (tc_w2 @ centers^T)[kc*128 + p, k]
    tcw2c = np.ascontiguousarray(
        (f("tc_w2") @ cen.T).reshape(8, P, K4).transpose(1, 0, 2))
    tb2c = np.ascontiguousarray(
        (f("tc_b2") @ cen.T).reshape(K4, 1))
    cgf = np.ascontiguousarray(
        np.stack([cen @ g_w[D:], cen @ fc_w1[D:]]))
    return {
        "wA_pack": wA, "w8_pack": w8, "bias_pack": bias,
        "tcw2c_pack": tcw2c, "tb2c_pack": tb2c, "cgf_pack": cgf,
    }


def _in_maps(inputs):
    shared = _pack_inputs(inputs)
    q = np.asarray(inputs["queries"], dtype=np.float32)
    return [dict(shared, queries=np.ascontiguousarray(q[c]))
            for c in range(B)]


def kernel(**inputs) -> np.ndarray:
    from concourse import bass_utils
    nc = _get_nc(reps=1)
    res = bass_utils.run_bass_kernel_spmd(nc, _in_maps(inputs),
                                          core_ids=list(range(B)))
    return np.stack([res.results[c]["out"] for c in range(B)], axis=0)


REPS = 100
NPIPE = 6


def kernel_timed(inputs, iters=3):
    """Returns (output [B,N,D], best_seconds_per_execution).

    Times a NEFF whose body is the full kernel repeated REPS times in a
    hardware loop; per-execution time = burst_wall / (NPIPE * REPS). This
    amortizes the axon-tunnel dispatch overhead (~1-80 ms per dispatch),
    which would otherwise dominate the measurement.
    """
    import jax
    from jax.sharding import Mesh, PartitionSpec, NamedSharding
    from jax.experimental.shard_map import shard_map
    from concourse import mybir
    from concourse.bass2jax import (_bass_exec_p, install_neuronx_cc_hook,
                                    partition_id_tensor)
    nc = _get_nc(reps=REPS)
    install_neuronx_cc_hook()
    partition_name = (nc.partition_id_tensor.name
                      if nc.partition_id_tensor else None)
    in_names, out_names, out_avals = [], [], []
    for alloc in nc.m.functions[0].allocations:
        if not isinstance(alloc, mybir.MemoryLocationSet):
            continue
        name = alloc.memorylocations[0].name
        if alloc.kind == "ExternalInput":
            if name != partition_name:
                in_names.append(name)
        elif alloc.kind == "ExternalOutput":
            out_names.append(name)
            out_avals.append(jax.core.ShapedArray(
                tuple(alloc.tensor_shape), mybir.dt.np(alloc.dtype)))

    all_in = list(in_names) + list(out_names)
    if partition_name is not None:
        all_in.append(partition_name)

    def _body(*args):
        operands = list(args)
        if partition_name is not None:
            operands.append(partition_id_tensor())
        return tuple(_bass_exec_p.bind(
            *operands, out_avals=tuple(out_avals), in_names=tuple(all_in),
            out_names=tuple(out_names), lowering_input_output_aliases=(),
            sim_require_finite=True, sim_require_nnan=True, nc=nc))

    devices = jax.devices()[:B]
    mesh = Mesh(np.asarray(devices), ("core",))
    n_par, n_out = len(in_names), len(out_names)
    fn = jax.jit(shard_map(_body, mesh=mesh,
                           in_specs=(PartitionSpec("core"),) * (n_par + n_out),
                           out_specs=(PartitionSpec("core"),) * n_out,
                           check_rep=False), keep_unused=True)
    sh = NamedSharding(mesh, PartitionSpec("core"))
    im = _in_maps(inputs)
    dev_args = [jax.device_put(
        np.concatenate([np.asarray(im[c][n]) for c in range(B)], axis=0), sh)
        for n in in_names]
    dev_zero = [jax.device_put(
        np.zeros((B * a.shape[0], *a.shape[1:]), a.dtype), sh)
        for a in out_avals]
    jax.block_until_ready(dev_args + dev_zero)
    outs = fn(*dev_args, *dev_zero)
    jax.block_until_ready(outs)
    best = None
    for burst in range(4):
        if burst:
            time.sleep(2.0)   # let the device cool between bursts
        t0 = time.perf_counter()
        last = None
        for _ in range(NPIPE):
            last = fn(*dev_args, *dev_zero)
        jax.block_until_ready(last)
        wall = time.perf_counter() - t0
        per = wall / (NPIPE * REPS)
        print(f"burst: {wall*1e3:.1f} ms / {NPIPE}x{REPS} execs "
              f"= {per*1e6:.1f} us/exec", flush=True)
        best = per if best is None else min(best, per)
    oi = out_names.index("out")
    full = np.asarray(outs[oi]).reshape(B, N, D)
    return full, best
